# revision 1
# baseline (speedup 1.0000x reference)
"""Trainium2 Bass kernel for nn_CovarianceLayer: local 5x5 covariance of two images.

reference:
    xc = x[:, :, 2:-2, 2:-2]; yc likewise
    x_dev = xc - box5x5(x)/25 ; y_dev = yc - box5x5(y)/25
    out   = box5x5(x_dev * y_dev)/25            # [B,1,1016,1016]

Strategy (pure data parallel over batch, 2 images per NeuronCore, 8 cores):
  Per 128-row block (stride 120, 8-row vertical halo):
    - horizontal 5-tap box sums on DVE via a custom prefix-scan op (1 pass):
        box5[k] = scan(ADD, v[k+4]-v[k], init=sum(v[0:4])) + v[k]
    - vertical 5-tap conv + center-crop subtraction fused into PE matmuls
      (f32r, full-rate): PSUM = Wid^T @ x_shift  -  (band/25)^T @ hx
    - p = x_dev * y_dev elementwise (DVE or GPSIMD)
    - horizontal box on p (DVE custom op), vertical band matmul (+1/25) on PE
    - PSUM -> SBUF copies on ScalarE, DMA in/out on SyncE (HWDGE)
"""

import numpy as np

import concourse.bacc as bacc
import concourse.mybir as mybir
import concourse.tile as tile
import concourse.dve_ops as dve_ops
from concourse.dve_spec import Spec, Src0, Src1, C0, scan, AluOp, lower
from concourse.dve_uop import DveOpSpec
from concourse.dve_ops import DveOp
from concourse import bass_utils

dt = mybir.dt

H = W = 1024
HO = WO = 1016   # output spatial dims
HD = WD = 1020   # x_dev dims
B_PER_CORE = 2
N_CORES = 8
BLK = 120        # output rows per block
MUL_ON_POOL = True


def _register_box5():
    """out[p,k] = sum_{d=0..4} v[p,k+d]; in0=v[:,4:4+N], in1=v[:,0:N], s0=sum(v[:,0:4])."""
    name = "BOX5_ANT"
    for op in dve_ops.OPS:
        if op.name == name:
            return op
    body = scan(AluOp.ADD, Src0 - Src1, init=C0) + Src1

    def ref(in0, in1, c0, c1, c2):
        return np.cumsum(in0 - in1, axis=-1, dtype=np.float32) + in1 + c0

    spec = Spec(body=body, reference=ref)
    row = dve_ops._CUSTOM_DVE_ROW_BASE + len(dve_ops.OPS)
    shas = {}
    for ver in ("v3", "v4"):
        uops = lower(spec, ver=ver)
        shas[ver] = DveOpSpec(name=name, opcode=row, uops=uops, rd1_en=True).sha(ver)
    op = DveOp(name, spec, subdim=False, uops_sha=shas)
    dve_ops.OPS.append(op)
    dve_ops.CUSTOM_DVE_SPECS[name] = spec
    dve_ops._SUB_OPCODE_FOR_NAME[name] = row
    return op


def _make_weights():
    # Wid[k, m] = 1 iff k == m+2         (center-crop tap, rides in the same PSUM group)
    # Wnb[k, m] = -1/25 iff m <= k <= m+4  (negated vertical band => PSUM = xc - mean)
    # Wpb[k, m] = +1/25 iff m <= k <= m+4  (final vertical band with the 1/25 scale)
    wid = np.zeros((128, 124), dtype=np.float32)
    wnb = np.zeros((128, 124), dtype=np.float32)
    for m in range(124):
        wid[m + 2, m] = 1.0
        wnb[m:m + 5, m] = -1.0 / 25.0
    wpb = np.zeros((124, 120), dtype=np.float32)
    for m in range(120):
        wpb[m:m + 5, m] = 1.0 / 25.0
    return wid, wnb, wpb


def _hsum_into(nc, box5, sb, out_tile, src_ap, rows, n_in, tag):
    """out_tile[0:rows, 0:n_in-4] = horizontal 5-tap box sums of src_ap[0:rows, 0:n_in]."""
    s3 = sb.tile([128, 1], dt.float32, tag=f"s3_{tag}")
    nc.vector.tensor_reduce(s3[0:rows, :], src_ap[0:rows, 0:4],
                            op=mybir.AluOpType.add, axis=mybir.AxisListType.X)
    n_out = n_in - 4
    nc.vector._custom_dve(box5, out=out_tile[0:rows, 0:n_out],
                          in0=src_ap[0:rows, 4:n_in],
                          in1=src_ap[0:rows, 0:n_out],
                          s0=s3[0:rows, :])


def build_bass():
    box5 = _register_box5()
    nc = bacc.Bacc("TRN2", target_bir_lowering=False)

    x_d = nc.dram_tensor("x", [B_PER_CORE, H, W], dt.float32r, kind="ExternalInput")
    y_d = nc.dram_tensor("y", [B_PER_CORE, H, W], dt.float32r, kind="ExternalInput")
    wid_d = nc.dram_tensor("wid", [128, 124], dt.float32r, kind="ExternalInput")
    wnb_d = nc.dram_tensor("wnb", [128, 124], dt.float32r, kind="ExternalInput")
    wpb_d = nc.dram_tensor("wpb", [124, 120], dt.float32r, kind="ExternalInput")
    o_d = nc.dram_tensor("o", [B_PER_CORE, HO, WO], dt.float32, kind="ExternalOutput")

    n_blocks = (HO + BLK - 1) // BLK  # 9

    with tile.TileContext(nc) as tc:
        with tc.tile_pool(name="wts", bufs=1) as wts, \
             tc.tile_pool(name="sb", bufs=3) as sb, \
             tc.tile_pool(name="ps_dev", bufs=1, space="PSUM") as ps_dev, \
             tc.tile_pool(name="ps_out", bufs=2, space="PSUM") as ps_out:

            wid_t = wts.tile([128, 124], dt.float32r)
            nc.sync.dma_start(wid_t[:], wid_d[:])
            wnb_t = wts.tile([128, 124], dt.float32r)
            nc.sync.dma_start(wnb_t[:], wnb_d[:])
            wpb_t = wts.tile([124, 120], dt.float32r)
            nc.sync.dma_start(wpb_t[:], wpb_d[:])

            for img in range(B_PER_CORE):
                for t in range(n_blocks):
                    r0 = BLK * t
                    rows = min(128, H - r0)          # input rows this block
                    d_rows = min(124, HD - r0)       # x_dev/p rows
                    o_rows = min(BLK, HO - r0)       # output rows

                    x_t = sb.tile([128, W], dt.float32r, tag="x_t")
                    nc.sync.dma_start(x_t[0:rows, :], x_d[img, r0:r0 + rows, :])
                    y_t = sb.tile([128, W], dt.float32r, tag="y_t")
                    nc.sync.dma_start(y_t[0:rows, :], y_d[img, r0:r0 + rows, :])

                    hx = sb.tile([128, WD], dt.float32r, tag="hx")
                    _hsum_into(nc, box5, sb, hx, x_t, rows, W, "x")
                    hy = sb.tile([128, WD], dt.float32r, tag="hy")
                    _hsum_into(nc, box5, sb, hy, y_t, rows, W, "y")

                    # PSUM = xc - box5x5(x)/25  (two col groups, bank-aligned)
                    xd_ps = ps_dev.tile([128, 1024], dt.float32, tag="xd_ps")
                    yd_ps = ps_dev.tile([128, 1024], dt.float32, tag="yd_ps")
                    for (dev_ps, src_t, hsrc) in ((xd_ps, x_t, hx), (yd_ps, y_t, hy)):
                        for c0, cn in ((0, 512), (512, WD - 512)):
                            nc.tensor.matmul(dev_ps[0:d_rows, c0:c0 + cn],
                                             lhsT=wid_t[0:rows, 0:d_rows],
                                             rhs=src_t[0:rows, 2 + c0:2 + c0 + cn],
                                             start=True, stop=False)
                            nc.tensor.matmul(dev_ps[0:d_rows, c0:c0 + cn],
                                             lhsT=wnb_t[0:rows, 0:d_rows],
                                             rhs=hsrc[0:rows, c0:c0 + cn],
                                             start=False, stop=True)

                    yd_s = sb.tile([128, WD], dt.float32, tag="yd_s")
                    nc.scalar.copy(yd_s[0:d_rows, :], yd_ps[0:d_rows, 0:WD])
                    p_s = sb.tile([128, WD], dt.float32, tag="p_s")
                    if MUL_ON_POOL:
                        xd_s = sb.tile([128, WD], dt.float32, tag="xd_s")
                        nc.scalar.copy(xd_s[0:d_rows, :], xd_ps[0:d_rows, 0:WD])
                        nc.gpsimd.tensor_mul(p_s[0:d_rows, :], xd_s[0:d_rows, :],
                                             yd_s[0:d_rows, :])
                    else:
                        nc.vector.tensor_mul(p_s[0:d_rows, :], xd_ps[0:d_rows, 0:WD],
                                             yd_s[0:d_rows, :])

                    hp = sb.tile([128, WO], dt.float32r, tag="hp")
                    _hsum_into(nc, box5, sb, hp, p_s, d_rows, WD, "p")

                    out_ps = ps_out.tile([128, 1024], dt.float32, tag="out_ps")
                    for c0, cn in ((0, 512), (512, WO - 512)):
                        nc.tensor.matmul(out_ps[0:o_rows, c0:c0 + cn],
                                         lhsT=wpb_t[0:d_rows, 0:o_rows],
                                         rhs=hp[0:d_rows, c0:c0 + cn],
                                         start=True, stop=True)

                    o_s = sb.tile([128, WO], dt.float32, tag="o_s")
                    nc.scalar.copy(o_s[0:o_rows, :], out_ps[0:o_rows, 0:WO])
                    nc.sync.dma_start(o_d[img, r0:r0 + o_rows, :], o_s[0:o_rows, :])

    nc.compile()
    return nc


_NC = None


def _get_nc():
    global _NC
    if _NC is None:
        _NC = build_bass()
    return _NC


def kernel(x: np.ndarray, y: np.ndarray, mean_mask: np.ndarray = None, *,
           trace: bool = False, **_ignored):
    """Full inputs x,y [16,1,1024,1024] f32 -> full output [16,1,1016,1016] f32."""
    assert x.shape == (16, 1, H, W) and y.shape == (16, 1, H, W)
    nc = _get_nc()
    wid, wnb, wpb = _make_weights()
    x4 = np.ascontiguousarray(x[:, 0], dtype=np.float32)
    y4 = np.ascontiguousarray(y[:, 0], dtype=np.float32)
    in_maps = []
    for c in range(N_CORES):
        in_maps.append({
            "x": x4[c * B_PER_CORE:(c + 1) * B_PER_CORE],
            "y": y4[c * B_PER_CORE:(c + 1) * B_PER_CORE],
            "wid": wid, "wnb": wnb, "wpb": wpb,
        })
    res = bass_utils.run_bass_kernel_spmd(nc, in_maps, core_ids=list(range(N_CORES)),
                                          trace=trace)
    out = np.concatenate([r["o"] for r in res.results], axis=0)
    kernel.last_results = res
    return out.reshape(16, 1, HO, WO)


# revision 2
# speedup vs baseline: 25042.0063x; 25042.0063x over previous
"""Trainium2 Bass kernel for nn_CovarianceLayer: local 5x5 covariance of two images.

reference:
    xc = x[:, :, 2:-2, 2:-2]; yc likewise
    x_dev = xc - box5x5(x)/25 ; y_dev = yc - box5x5(y)/25
    out   = box5x5(x_dev * y_dev)/25            # [B,1,1016,1016]

Strategy (pure data parallel over batch, 2 images per NeuronCore, 8 cores):
  Per 128-row block (stride 120, 8-row vertical halo):
    - horizontal 5-tap box sums on DVE via a custom prefix-scan op (1 pass):
        box5[k] = scan(ADD, v[k+4]-v[k], init=sum(v[0:4])) + v[k]
    - vertical 5-tap conv + center-crop subtraction fused into PE matmuls
      (f32r, full-rate): PSUM = Wid^T @ x_shift  -  (band/25)^T @ hx
    - p = x_dev * y_dev elementwise (DVE or GPSIMD)
    - horizontal box on p (DVE custom op), vertical band matmul (+1/25) on PE
    - PSUM -> SBUF copies on ScalarE, DMA in/out on SyncE (HWDGE)
"""

import numpy as np

import concourse.bacc as bacc
import concourse.mybir as mybir
import concourse.tile as tile
import concourse.dve_ops as dve_ops
from concourse.dve_spec import Spec, Src0, Src1, C0, scan, AluOp, lower
from concourse.dve_uop import DveOpSpec
from concourse.dve_ops import DveOp
from concourse import bass_utils

dt = mybir.dt

H = W = 1024
HO = WO = 1016   # output spatial dims
HD = WD = 1020   # x_dev dims
B_PER_CORE = 2
N_CORES = 8
BLK = 120        # output rows per block
MUL_ON_POOL = True


def _register_box5():
    """out[p,k] = sum_{d=0..4} v[p,k+d]; in0=v[:,4:4+N], in1=v[:,0:N], s0=sum(v[:,0:4])."""
    name = "BOX5_ANT"
    for op in dve_ops.OPS:
        if op.name == name:
            return op
    body = scan(AluOp.ADD, Src0 - Src1, init=C0) + Src1

    def ref(in0, in1, c0, c1, c2):
        return np.cumsum(in0 - in1, axis=-1, dtype=np.float32) + in1 + c0

    spec = Spec(body=body, reference=ref)
    row = dve_ops._CUSTOM_DVE_ROW_BASE + len(dve_ops.OPS)
    shas = {}
    for ver in ("v3", "v4"):
        uops = lower(spec, ver=ver)
        shas[ver] = DveOpSpec(name=name, opcode=row, uops=uops, rd1_en=True).sha(ver)
    op = DveOp(name, spec, subdim=False, uops_sha=shas)
    dve_ops.OPS.append(op)
    dve_ops.CUSTOM_DVE_SPECS[name] = spec
    dve_ops._SUB_OPCODE_FOR_NAME[name] = row
    return op


def _make_weights():
    # Wid[k, m] = 1 iff k == m+2         (center-crop tap, rides in the same PSUM group)
    # Wnb[k, m] = -1/25 iff m <= k <= m+4  (negated vertical band => PSUM = xc - mean)
    # Wpb[k, m] = +1/25 iff m <= k <= m+4  (final vertical band with the 1/25 scale)
    wid = np.zeros((128, 124), dtype=np.float32)
    wnb = np.zeros((128, 124), dtype=np.float32)
    for m in range(124):
        wid[m + 2, m] = 1.0
        wnb[m:m + 5, m] = -1.0 / 25.0
    wpb = np.zeros((124, 120), dtype=np.float32)
    for m in range(120):
        wpb[m:m + 5, m] = 1.0 / 25.0
    return wid, wnb, wpb


def _hsum_into(nc, box5, sb, out_tile, src_ap, rows, n_in, tag):
    """out_tile[0:rows, 0:n_in-4] = horizontal 5-tap box sums of src_ap[0:rows, 0:n_in]."""
    s3 = sb.tile([128, 1], dt.float32, tag=f"s3_{tag}")
    nc.vector.tensor_reduce(s3[0:rows, :], src_ap[0:rows, 0:4],
                            op=mybir.AluOpType.add, axis=mybir.AxisListType.X)
    n_out = n_in - 4
    nc.vector._custom_dve(box5, out=out_tile[0:rows, 0:n_out],
                          in0=src_ap[0:rows, 4:n_in],
                          in1=src_ap[0:rows, 0:n_out],
                          s0=s3[0:rows, :])


def build_bass():
    box5 = _register_box5()
    nc = bacc.Bacc("TRN2", target_bir_lowering=False)

    x_d = nc.dram_tensor("x", [B_PER_CORE, H, W], dt.float32r, kind="ExternalInput")
    y_d = nc.dram_tensor("y", [B_PER_CORE, H, W], dt.float32r, kind="ExternalInput")
    wid_d = nc.dram_tensor("wid", [128, 124], dt.float32r, kind="ExternalInput")
    wnb_d = nc.dram_tensor("wnb", [128, 124], dt.float32r, kind="ExternalInput")
    wpb_d = nc.dram_tensor("wpb", [124, 120], dt.float32r, kind="ExternalInput")
    o_d = nc.dram_tensor("o", [B_PER_CORE, HO, WO], dt.float32, kind="ExternalOutput")

    n_blocks = (HO + BLK - 1) // BLK  # 9

    with tile.TileContext(nc) as tc:
        with tc.tile_pool(name="wts", bufs=1) as wts, \
             tc.tile_pool(name="sb", bufs=3) as sb, \
             tc.tile_pool(name="ps_dev", bufs=1, space="PSUM") as ps_dev, \
             tc.tile_pool(name="ps_out", bufs=2, space="PSUM") as ps_out:

            wid_t = wts.tile([128, 124], dt.float32r)
            nc.sync.dma_start(wid_t[:], wid_d[:])
            wnb_t = wts.tile([128, 124], dt.float32r)
            nc.sync.dma_start(wnb_t[:], wnb_d[:])
            wpb_t = wts.tile([124, 120], dt.float32r)
            nc.sync.dma_start(wpb_t[:], wpb_d[:])

            for img in range(B_PER_CORE):
                for t in range(n_blocks):
                    r0 = BLK * t
                    rows = min(128, H - r0)          # input rows this block
                    d_rows = min(124, HD - r0)       # x_dev/p rows
                    o_rows = min(BLK, HO - r0)       # output rows

                    x_t = sb.tile([128, W], dt.float32r, tag="x_t")
                    nc.sync.dma_start(x_t[0:rows, :], x_d[img, r0:r0 + rows, :])
                    y_t = sb.tile([128, W], dt.float32r, tag="y_t")
                    nc.sync.dma_start(y_t[0:rows, :], y_d[img, r0:r0 + rows, :])

                    hx = sb.tile([128, WD], dt.float32r, tag="hx")
                    _hsum_into(nc, box5, sb, hx, x_t, rows, W, "x")
                    hy = sb.tile([128, WD], dt.float32r, tag="hy")
                    _hsum_into(nc, box5, sb, hy, y_t, rows, W, "y")

                    # PSUM = xc - box5x5(x)/25  (two col groups, bank-aligned)
                    xd_ps = ps_dev.tile([128, 1024], dt.float32, tag="xd_ps")
                    yd_ps = ps_dev.tile([128, 1024], dt.float32, tag="yd_ps")
                    for (dev_ps, src_t, hsrc) in ((xd_ps, x_t, hx), (yd_ps, y_t, hy)):
                        for c0, cn in ((0, 512), (512, WD - 512)):
                            nc.tensor.matmul(dev_ps[0:d_rows, c0:c0 + cn],
                                             lhsT=wid_t[0:rows, 0:d_rows],
                                             rhs=src_t[0:rows, 2 + c0:2 + c0 + cn],
                                             start=True, stop=False)
                            nc.tensor.matmul(dev_ps[0:d_rows, c0:c0 + cn],
                                             lhsT=wnb_t[0:rows, 0:d_rows],
                                             rhs=hsrc[0:rows, c0:c0 + cn],
                                             start=False, stop=True)

                    yd_s = sb.tile([128, WD], dt.float32, tag="yd_s")
                    nc.scalar.copy(yd_s[0:d_rows, :], yd_ps[0:d_rows, 0:WD])
                    p_s = sb.tile([128, WD], dt.float32, tag="p_s")
                    if MUL_ON_POOL:
                        xd_s = sb.tile([128, WD], dt.float32, tag="xd_s")
                        nc.scalar.copy(xd_s[0:d_rows, :], xd_ps[0:d_rows, 0:WD])
                        nc.gpsimd.tensor_mul(p_s[0:d_rows, :], xd_s[0:d_rows, :],
                                             yd_s[0:d_rows, :])
                    else:
                        nc.vector.tensor_mul(p_s[0:d_rows, :], xd_ps[0:d_rows, 0:WD],
                                             yd_s[0:d_rows, :])

                    hp = sb.tile([128, WO], dt.float32r, tag="hp")
                    _hsum_into(nc, box5, sb, hp, p_s, d_rows, WD, "p")

                    out_ps = ps_out.tile([128, 1024], dt.float32, tag="out_ps")
                    for c0, cn in ((0, 512), (512, WO - 512)):
                        nc.tensor.matmul(out_ps[0:o_rows, c0:c0 + cn],
                                         lhsT=wpb_t[0:d_rows, 0:o_rows],
                                         rhs=hp[0:d_rows, c0:c0 + cn],
                                         start=True, stop=True)

                    o_s = sb.tile([128, WO], dt.float32, tag="o_s")
                    nc.scalar.copy(o_s[0:o_rows, :], out_ps[0:o_rows, 0:WO])
                    nc.sync.dma_start(o_d[img, r0:r0 + o_rows, :], o_s[0:o_rows, :])

    nc.compile()
    return nc


_NC = None


def _get_nc():
    global _NC
    if _NC is None:
        _NC = build_bass()
    return _NC


def kernel(x: np.ndarray, y: np.ndarray, mean_mask: np.ndarray = None, *,
           trace: bool = False, **_ignored):
    """Full inputs x,y [16,1,1024,1024] f32 -> full output [16,1,1016,1016] f32."""
    assert x.shape == (16, 1, H, W) and y.shape == (16, 1, H, W)
    nc = _get_nc()
    wid, wnb, wpb = _make_weights()
    x4 = np.ascontiguousarray(x[:, 0], dtype=np.float32)
    y4 = np.ascontiguousarray(y[:, 0], dtype=np.float32)
    in_maps = []
    for c in range(N_CORES):
        in_maps.append({
            "x": x4[c * B_PER_CORE:(c + 1) * B_PER_CORE],
            "y": y4[c * B_PER_CORE:(c + 1) * B_PER_CORE],
            "wid": wid, "wnb": wnb, "wpb": wpb,
        })
    kw = {}
    if trace:
        kw = dict(trace=True, trace_cores=[0])
    res = bass_utils.run_bass_kernel_spmd(nc, in_maps, core_ids=list(range(N_CORES)),
                                          **kw)
    out = np.concatenate([r["o"] for r in res.results], axis=0)
    kernel.last_results = res
    return out.reshape(16, 1, HO, WO)
